# revision 9
# baseline (speedup 1.0000x reference)
"""Trainium2 Bass kernel for nn_AudioDeviceModel (18-layer dilated causal CNN).

Data parallel over batch (64 = 8 cores x 8).  Per core, (batch, chan) packs
the 128 SBUF partitions and time is the free dim; each conv tap is one
block-diagonal [128,128]x[128,w] TensorEngine matmul with dilation shifts as
free-dim offsets.

The 1x1 "io" mix and the halved channel-sum skip are folded away algebraically:
  sig_{i+1} = io_i(h_i) + io_b_i + S_i,   S_i = sum_c(sig_i)/2
so conv_{i+1} applied to sig_{i+1} becomes composed taps (io_i @ W_k) read
directly from h_i, plus a 1-channel S path with
  S_{i+1} = u_i . h_i + 8 S_i + c_i,  u_i = io_w_i.sum(1)/2.
Constants migrate into conv biases via gamma_{i+1} = 8 gamma_i + c_i.  The S
taps ride in the ctrl matmul (96 contraction rows: 72 ctrl + 24 S-im2col), the
S recursion is one vector op per tile, and the S im2col shifts are SBUF->SBUF
DMA copies issued from the scalar engine's DGE queue.  This removes 2 of 7
matmuls per tile vs the direct formulation.  Epilogue matmuls (S-update, mixer)
lag the conv stream by one tile so ReLU eviction latency stays off the PE
critical path.
"""

import numpy as np
import ml_dtypes

import concourse.bass as bass
import concourse.tile as tile
from concourse import bacc, mybir
from concourse.bass_utils import run_bass_kernel_spmd

# Problem constants (hardcoded; kernel.py must be self-contained).
DILATIONS = [1, 2, 4, 8, 16, 32, 64, 128, 256, 1, 2, 4, 8, 16, 32, 64, 128, 256]
UNIQ_DIL = [1, 2, 4, 8, 16, 32, 64, 128, 256]
DI = [UNIQ_DIL.index(d) for d in DILATIONS]
KSIZE = 3
CH = 16
NUM_SIG = 1
NUM_CTRL = 3
FRAME = 2048
T = 4092
B = 64
NCORES = 8
BL = B // NCORES          # 8 batches per core
W = T                     # no left pad needed: trim bounds never read t<0
NL = len(DILATIONS)       # 18
TT = 512                  # time tile
NTILES = (T + TT - 1) // TT   # 8 (last tile 508 wide)
MIX_T0 = T - FRAME        # 2044: first time index contributing to output

# Receptive-field trim: layer i's output h_i only influences the final frame
# for t >= LO[i]; LO[i] = max(0, LO[i+1] - 2*d_{i+1}) with LO[17] = MIX_T0.
_sums = [0] * NL
_acc = 0
for _i in range(NL - 1, -1, -1):
    _sums[_i] = _acc
    _acc += DILATIONS[_i]
LO = [max(0, MIX_T0 - 2 * _sums[_i]) for _i in range(NL)]

BF16 = ml_dtypes.bfloat16

# Weight bank column layout (bf16, [128, NW]), in DMA-stream order:
#   0:384     layer-0 taps (rows :32)
#   384:392   u_0
#   392:536   mixer blocks (18 x 8)
#   base(i) = 536 + (i-1)*520 for i=1..15: taps(384) | ctrlS(128, rows :104) | u_i(8)
#   i=16,17: 512-wide blocks (no u)
def _base(i):
    assert i >= 1
    return 536 + (i - 1) * 520 if i <= 16 else 536 + 15 * 520 + 512

NW = 536 + 15 * 520 + 512 + 512       # 9360
O_U0 = 384
O_MIX = 392
NBIAS = NL + 1                        # 18 conv biases | mixer_b


def _bd(block, k_per_b):
    # block: [k_per_b, 16] -> [8*k_per_b, 128] block diagonal over batches
    m = np.zeros((8 * k_per_b, 128), np.float32)
    for b in range(8):
        m[b * k_per_b:(b + 1) * k_per_b, b * 16:(b + 1) * 16] = block
    return m


def _build_weight_bank(conv_w0, conv_w, conv_b, io_w, io_b, mixer_w, mixer_b):
    conv_w0 = conv_w0.astype(np.float32)
    conv_w = conv_w.astype(np.float32)
    conv_b = conv_b.astype(np.float32)
    io_w = io_w.astype(np.float32)
    io_b = io_b.astype(np.float32)
    mixer_w = mixer_w.astype(np.float32)
    mixer_b = mixer_b.astype(np.float32)

    wbank = np.zeros((128, NW), np.float32)
    bbank = np.zeros((128, NBIAS), np.float32)

    # gamma_i: S_i = S~_i + gamma_i
    gam = [0.0] * NL
    for i in range(1, NL):
        gam[i] = 8.0 * gam[i - 1] + io_b[i - 1].sum() / 2.0

    # layer 0: raw taps on [sig|ctrl] (4 ch/batch)
    for k in range(KSIZE):
        wbank[:32, k * 128:(k + 1) * 128] = _bd(conv_w0[k], 4)
    bbank[:, 0] = np.tile(conv_b[0], 8)

    # u_i blocks (i = 0..15)
    for i in range(16):
        u = io_w[i].sum(axis=1) / 2.0          # [16]
        o = O_U0 if i == 0 else _base(i) + 512
        for b in range(8):
            wbank[b * 16:(b + 1) * 16, o + b] = u

    # mixer blocks
    for i in range(NL):
        for b in range(8):
            wbank[b * 16:(b + 1) * 16, O_MIX + i * 8 + b] = \
                mixer_w[i * CH:(i + 1) * CH, 0]
    bbank[:8, NL] = mixer_b[0]

    # layers 1..17: composed taps + ctrlS block + bias-hat
    for i in range(1, NL):
        wk = conv_w[i - 1]                     # [K, 19, 16]
        o = _base(i)
        bias = conv_b[i].copy()
        vsum = np.zeros(CH, np.float32)
        for k in range(KSIZE):
            comp = io_w[i - 1] @ wk[k][:CH]    # [16(h), 16(out)]
            wbank[:, o + k * 128: o + (k + 1) * 128] = _bd(comp, CH)
            bias += io_b[i - 1] @ wk[k][:CH]
            vsum += wk[k][:CH].sum(axis=0)
        bias += vsum * gam[i - 1]
        bbank[:, i] = np.tile(bias, 8)
        # ctrlS rows: 0-7 S canonical (k=2), 8-15 k=1, 16-23 k=0,
        # 24-31 zero, 32-103 ctrl (32 + b*9 + k*3 + c)
        blk = np.zeros((104, 128), np.float32)
        for b in range(8):
            for k in range(KSIZE):
                vk = wk[k][:CH].sum(axis=0)    # [16]
                for c in range(NUM_CTRL):
                    blk[32 + b * 9 + k * 3 + c, b * 16:(b + 1) * 16] = wk[k][CH + c]
                blk[(KSIZE - 1 - k) * 8 + b, b * 16:(b + 1) * 16] = vk
        wbank[:104, o + 384: o + 512] = blk

    return wbank.astype(BF16), bbank


def _build_per_core_inputs(x_core):
    """x_core: [BL, T, 4] f32 -> (x0 [96, W] bf16, ctrlb [104, 9*W] bf16)."""
    x_core = x_core.astype(np.float32)
    x0 = np.zeros((3 * BL * 4, W), np.float32)
    for b in range(BL):
        x0[b * 4:(b + 1) * 4, :] = x_core[b].T
    x0[32:64] = x0[:32]
    x0[64:96] = x0[:32]
    ctrl = x_core[:, :, NUM_SIG:]  # [BL, T, 3]
    ctrlb = np.zeros((104, len(UNIQ_DIL) * W), np.float32)
    for di, d in enumerate(UNIQ_DIL):
        for k in range(KSIZE):
            shift = (KSIZE - 1 - k) * d
            for c in range(NUM_CTRL):
                for b in range(BL):
                    r = 32 + b * 9 + k * 3 + c
                    ctrlb[r, di * W + shift: di * W + T] = ctrl[b, : T - shift, c]
    # S~_0 im2col for layer 1 (d=2, block 1), canonical copy in rows 88-95
    s0 = x_core[:, :, 0] / 2.0
    d1 = DILATIONS[1]
    for k in range(KSIZE):
        shift = (KSIZE - 1 - k) * d1
        for b in range(BL):
            ctrlb[(KSIZE - 1 - k) * 8 + b, 1 * W + shift: 1 * W + T] = s0[b, : T - shift]
    return x0.astype(BF16), ctrlb.astype(BF16)


def build_graph():
    nc = bacc.Bacc("TRN2", target_bir_lowering=False, debug=False)

    p_x0 = nc.declare_dram_parameter("x0", [96, W], mybir.dt.bfloat16, isOutput=False)
    p_ctrl = nc.declare_dram_parameter(
        "ctrlb", [104, len(UNIQ_DIL) * W], mybir.dt.bfloat16, isOutput=False)
    p_w = nc.declare_dram_parameter("wbank", [128, NW], mybir.dt.bfloat16, isOutput=False)
    p_b = nc.declare_dram_parameter("bbank", [128, NBIAS], mybir.dt.float32, isOutput=False)
    p_out = nc.declare_dram_parameter("out", [8, FRAME], mybir.dt.float32, isOutput=True)

    with tile.TileContext(nc) as tc:
        with (
            tc.tile_pool(name="persist", bufs=1) as persist,
            tc.tile_pool(name="wu", bufs=1) as wu,
            tc.tile_pool(name="ps", bufs=2, space="PSUM") as ps,
            tc.tile_pool(name="sps", bufs=2, space="PSUM") as sps,
            tc.tile_pool(name="mixp", bufs=1, space="PSUM") as mixp,
        ):
            x0_sb = persist.tile([96, W], mybir.dt.bfloat16, tag="x0")
            ctrl_sb = persist.tile([104, len(UNIQ_DIL) * W], mybir.dt.bfloat16, tag="ctrl")
            w_sb = persist.tile([128, NW], mybir.dt.bfloat16, tag="wbank")
            b_sb = persist.tile([128, NBIAS], mybir.dt.float32, tag="bbank")
            hA = persist.tile([128, W], mybir.dt.bfloat16, tag="hA")
            hB = persist.tile([128, W], mybir.dt.bfloat16, tag="hB")
            out_sb = persist.tile([8, FRAME], mybir.dt.float32, tag="outsb")

            # Front DMAs on the sync HWDGE ring, in deadline order.  Block 1's
            # S rows ship first (layer 0's S-eviction reads them); ctrl blocks
            # for layers 2..9 are loaded mid-graph from the gpsimd queue, after
            # the device S writes to the same block have been emitted.
            nc.sync.dma_start(out=x0_sb[:], in_=p_x0[:])
            nc.sync.dma_start(out=w_sb[:, :392], in_=p_w[:, :392])
            nc.sync.dma_start(out=b_sb[:], in_=p_b[:])
            nc.sync.dma_start(out=ctrl_sb[:24, W:2 * W], in_=p_ctrl[:24, W:2 * W])
            nc.sync.dma_start(out=w_sb[:, 392:1056], in_=p_w[:, 392:1056])
            for i in (2, 3):
                o = _base(i)
                nc.sync.dma_start(out=w_sb[:, o:o + 520], in_=p_w[:, o:o + 520])
            nc.sync.dma_start(out=ctrl_sb[24:104, W:2 * W], in_=p_ctrl[24:104, W:2 * W])
            for i in range(4, NL):
                o = _base(i)
                wid = 520 if i <= 15 else 512
                nc.sync.dma_start(out=w_sb[:, o:o + wid], in_=p_w[:, o:o + wid])

            mixS = mixp.tile([8, FRAME], mybir.dt.float32, tag="mixS")

            # PE warm-up on x0 data (finite, lands first): ramps the PE
            # p-state/HAM clock before layer 0 without a vector dependency.
            for _ in range(6):
                wps = ps.tile([128, TT], mybir.dt.float32, tag="hps")
                nc.tensor.matmul(wps[:, :TT], x0_sb[:32, :128],
                                 x0_sb[:32, :TT], start=True, stop=True)

            pending = []

            def flush(keep=0):
                while len(pending) > keep:
                    pending.pop(0)()

            def o_tap(i, k):
                return k * 128 if i == 0 else _base(i) + k * 128

            for i in range(NL):
                d = DILATIONS[i]
                src_h = x0_sb if i == 0 else (hA if (i - 1) % 2 == 0 else hB)
                dst_h = hA if i % 2 == 0 else hB
                krows = 32 if i == 0 else 128
                lo = LO[i]
                for j in range(lo // TT, NTILES):
                    a = max(j * TT, lo)
                    b = min((j + 1) * TT, T)
                    wid = b - a
                    h_ps = ps.tile([128, TT], mybir.dt.float32, tag="hps")
                    if i == 0:
                        for k in range(KSIZE):
                            shift = (KSIZE - 1 - k) * d
                            nc.tensor.matmul(
                                h_ps[:, :wid],
                                w_sb[:32, k * 128:(k + 1) * 128],
                                x0_sb[:32, a - shift: a - shift + wid],
                                start=(k == 0),
                                stop=(k == KSIZE - 1),
                            )
                    else:
                        for k in range(KSIZE):
                            shift = (KSIZE - 1 - k) * d
                            nc.tensor.matmul(
                                h_ps[:, :wid],
                                w_sb[:, o_tap(i, k): o_tap(i, k) + 128],
                                src_h[:, a - shift: a - shift + wid],
                                start=(k == 0),
                                stop=False,
                            )
                    if i > 0:
                        c0 = _base(i) + 384
                        nc.tensor.matmul(
                            h_ps[:, :wid],
                            w_sb[:104, c0:c0 + 128],
                            ctrl_sb[:104, DI[i] * W + a: DI[i] * W + a + wid],
                            start=False,
                            stop=True,
                        )
                    nc.scalar.activation(
                        out=dst_h[:, a: a + wid],
                        in_=h_ps[:, :wid],
                        func=mybir.ActivationFunctionType.Relu,
                        bias=b_sb[:, i:i + 1],
                        scale=1.0,
                    )
                    flush(keep=2)

                    def epilogue(i=i, j=j, a=a, b=b, dst_h=dst_h):
                        # S-update: S~_{i+1}[t] = u_i.h_i[t] (+ 8 S~_i at evict)
                        if i <= 15 and b > LO[i + 1]:
                            sa = max(a, LO[i + 1])
                            sw = b - sa
                            so = sa - j * TT
                            o_u = O_U0 if i == 0 else _base(i) + 512
                            s_ps = sps.tile([40, TT], mybir.dt.float32, tag="sps")
                            nc.tensor.matmul(
                                s_ps[32:40, so: so + sw],
                                w_sb[:, o_u: o_u + 8],
                                dst_h[:, sa: sa + sw],
                                start=True, stop=True,
                                skip_group_check=True,
                            )
                        if b > MIX_T0:
                            ma = max(a, MIX_T0)
                            mw = b - ma
                            nc.tensor.matmul(
                                mixS[0:8, ma - MIX_T0: ma - MIX_T0 + mw],
                                w_sb[:, O_MIX + i * 8: O_MIX + i * 8 + 8],
                                dst_h[:, ma: ma + mw],
                                start=(i == 0),
                                stop=(i == NL - 1),
                                skip_group_check=True,
                            )
                            # stream the final eviction behind layer 17's
                            # last mix matmuls (chunk complete => evict)
                            if i == NL - 1 and j in (5, 6, 7):
                                c0, c1 = {5: (0, 1024), 6: (1024, 1536),
                                          7: (1536, 2048)}[j]
                                if j == 6:
                                    nc.vector.tensor_scalar_add(
                                        out=out_sb[:, c0:c1],
                                        in0=mixS[0:8, c0:c1],
                                        scalar1=b_sb[:8, NL: NL + 1],
                                    )
                                else:
                                    nc.scalar.activation(
                                        out=out_sb[:, c0:c1],
                                        in_=mixS[0:8, c0:c1],
                                        func=mybir.ActivationFunctionType.Identity,
                                        bias=b_sb[:8, NL: NL + 1],
                                        scale=1.0,
                                    )
                        if i <= 15 and b > LO[i + 1]:
                            sblk = DI[i + 1] * W
                            dblk = DI[i + 2] * W
                            nc.vector.scalar_tensor_tensor(
                                out=ctrl_sb[0:8, dblk + sa: dblk + sa + sw],
                                in0=ctrl_sb[0:8, sblk + sa: sblk + sa + sw],
                                scalar=8.0,
                                in1=s_ps[32:40, so: so + sw],
                                op0=mybir.AluOpType.mult,
                                op1=mybir.AluOpType.add,
                            )

                    pending.append(epilogue)

                if i <= 15:
                    # im2col shift copies of S~_{i+1} for layer i+2's taps,
                    # on the gpsimd engine's DGE queue (separate from sync's).
                    def im2col(i=i):
                        d2 = DILATIONS[i + 2]
                        dblk = DI[i + 2] * W
                        lo2 = LO[i + 2]
                        for k in range(2):
                            shift = (KSIZE - 1 - k) * d2
                            r0 = (KSIZE - 1 - k) * 8
                            nc.gpsimd.dma_start(
                                out=ctrl_sb[r0: r0 + 8,
                                            dblk + lo2: dblk + T],
                                in_=ctrl_sb[0:8,
                                            dblk + lo2 - shift: dblk + T - shift],
                            )

                    pending.append(im2col)
                    if i <= 7:
                        def ctrl_bulk(i=i):
                            di = DI[i + 2]
                            nc.gpsimd.dma_start(
                                out=ctrl_sb[24:104, di * W:(di + 1) * W],
                                in_=p_ctrl[24:104, di * W:(di + 1) * W])

                        pending.append(ctrl_bulk)

            flush()

            # out DMAs (evictions were streamed inside layer-17 epilogues)
            nc.sync.dma_start(out=p_out[:, :1536], in_=out_sb[:, :1536])
            nc.sync.dma_start(out=p_out[:, 1536:], in_=out_sb[:, 1536:])

    nc.finalize()
    return nc


_CACHE = {}


def kernel(**inputs) -> np.ndarray:
    inp = inputs["input"].astype(np.float32)          # [64, 4092, 4]
    wbank, bbank = _build_weight_bank(
        inputs["conv_w0"], inputs["conv_w"], inputs["conv_b"],
        inputs["io_w"], inputs["io_b"], inputs["mixer_w"], inputs["mixer_b"],
    )

    if "nc" not in _CACHE:
        _CACHE["nc"] = build_graph()
    nc = _CACHE["nc"]

    in_maps = []
    for c in range(NCORES):
        x0, ctrlb = _build_per_core_inputs(inp[c * BL:(c + 1) * BL])
        in_maps.append({"x0": x0, "ctrlb": ctrlb, "wbank": wbank, "bbank": bbank})

    res = run_bass_kernel_spmd(nc, in_maps, core_ids=list(range(NCORES)))
    outs = [res.results[c]["out"] for c in range(NCORES)]       # each [8, 2048]
    full = np.concatenate(outs, axis=0)                         # [64, 2048]
    return full[:, :, None].astype(np.float32)                  # [64, 2048, 1]


if __name__ == "__main__":
    data = np.load("/root/problem/ref_inputs.npz")
    out = kernel(**{k: data[k] for k in data.files})
    ref = np.load("/root/problem/ref_out.npy")
    err = np.linalg.norm(out - ref) / np.linalg.norm(ref)
    print("Relative error:", err)


# revision 10
# speedup vs baseline: 1.1724x; 1.1724x over previous
"""Trainium2 Bass kernel for nn_AudioDeviceModel (18-layer dilated causal CNN).

Data parallel over batch (64 = 8 cores x 8).  Per core, (batch, chan) packs
the 128 SBUF partitions and time is the free dim; each conv tap is one
block-diagonal [128,128]x[128,w] TensorEngine matmul with dilation shifts as
free-dim offsets.

The 1x1 "io" mix and the halved channel-sum skip are folded away algebraically:
  sig_{i+1} = io_i(h_i) + io_b_i + S_i,   S_i = sum_c(sig_i)/2
so conv_{i+1} applied to sig_{i+1} becomes composed taps (io_i @ W_k) read
directly from h_i, plus a 1-channel S path with
  S_{i+1} = u_i . h_i + 8 S_i + c_i,  u_i = io_w_i.sum(1)/2.
Constants migrate into conv biases via gamma_{i+1} = 8 gamma_i + c_i.  The S
taps ride in the ctrl matmul (96 contraction rows: 72 ctrl + 24 S-im2col), the
S recursion is one vector op per tile, and the S im2col shifts are SBUF->SBUF
DMA copies issued from the scalar engine's DGE queue.  This removes 2 of 7
matmuls per tile vs the direct formulation.  Epilogue matmuls (S-update, mixer)
lag the conv stream by one tile so ReLU eviction latency stays off the PE
critical path.
"""

import numpy as np
import ml_dtypes

import concourse.bass as bass
import concourse.tile as tile
from concourse import bacc, mybir
from concourse.bass_utils import run_bass_kernel_spmd

# Problem constants (hardcoded; kernel.py must be self-contained).
DILATIONS = [1, 2, 4, 8, 16, 32, 64, 128, 256, 1, 2, 4, 8, 16, 32, 64, 128, 256]
UNIQ_DIL = [1, 2, 4, 8, 16, 32, 64, 128, 256]
DI = [UNIQ_DIL.index(d) for d in DILATIONS]
KSIZE = 3
CH = 16
NUM_SIG = 1
NUM_CTRL = 3
FRAME = 2048
T = 4092
B = 64
NCORES = 8
BL = B // NCORES          # 8 batches per core
W = T                     # no left pad needed: trim bounds never read t<0
NL = len(DILATIONS)       # 18
TT = 512                  # time tile
NTILES = (T + TT - 1) // TT   # 8 (last tile 508 wide)
MIX_T0 = T - FRAME        # 2044: first time index contributing to output

# Receptive-field trim: layer i's output h_i only influences the final frame
# for t >= LO[i]; LO[i] = max(0, LO[i+1] - 2*d_{i+1}) with LO[17] = MIX_T0.
_sums = [0] * NL
_acc = 0
for _i in range(NL - 1, -1, -1):
    _sums[_i] = _acc
    _acc += DILATIONS[_i]
LO = [max(0, MIX_T0 - 2 * _sums[_i]) for _i in range(NL)]

BF16 = ml_dtypes.bfloat16

# Weight bank column layout (bf16, [128, NW]), in DMA-stream order:
#   0:384     layer-0 taps (rows :32)
#   384:392   u_0
#   392:536   mixer blocks (18 x 8)
#   base(i) = 536 + (i-1)*520 for i=1..15: taps(384) | ctrlS(128, rows :104) | u_i(8)
#   i=16,17: 512-wide blocks (no u)
def _base(i):
    assert i >= 1
    return 536 + (i - 1) * 520 if i <= 16 else 536 + 15 * 520 + 512

NW = 536 + 15 * 520 + 512 + 512       # 9360
O_U0 = 384
O_MIX = 392
NBIAS = NL + 1                        # 18 conv biases | mixer_b


def _bd(block, k_per_b):
    # block: [k_per_b, 16] -> [8*k_per_b, 128] block diagonal over batches
    m = np.zeros((8 * k_per_b, 128), np.float32)
    for b in range(8):
        m[b * k_per_b:(b + 1) * k_per_b, b * 16:(b + 1) * 16] = block
    return m


def _build_weight_bank(conv_w0, conv_w, conv_b, io_w, io_b, mixer_w, mixer_b):
    conv_w0 = conv_w0.astype(np.float32)
    conv_w = conv_w.astype(np.float32)
    conv_b = conv_b.astype(np.float32)
    io_w = io_w.astype(np.float32)
    io_b = io_b.astype(np.float32)
    mixer_w = mixer_w.astype(np.float32)
    mixer_b = mixer_b.astype(np.float32)

    wbank = np.zeros((128, NW), np.float32)
    bbank = np.zeros((128, NBIAS), np.float32)

    # gamma_i: S_i = S~_i + gamma_i
    gam = [0.0] * NL
    for i in range(1, NL):
        gam[i] = 8.0 * gam[i - 1] + io_b[i - 1].sum() / 2.0

    # layer 0: raw taps on [sig|ctrl] (4 ch/batch)
    for k in range(KSIZE):
        wbank[:32, k * 128:(k + 1) * 128] = _bd(conv_w0[k], 4)
    bbank[:, 0] = np.tile(conv_b[0], 8)

    # u_i blocks (i = 0..15)
    for i in range(16):
        u = io_w[i].sum(axis=1) / 2.0          # [16]
        o = O_U0 if i == 0 else _base(i) + 512
        for b in range(8):
            wbank[b * 16:(b + 1) * 16, o + b] = u

    # mixer blocks
    for i in range(NL):
        for b in range(8):
            wbank[b * 16:(b + 1) * 16, O_MIX + i * 8 + b] = \
                mixer_w[i * CH:(i + 1) * CH, 0]
    bbank[:8, NL] = mixer_b[0]

    # layers 1..17: composed taps + ctrlS block + bias-hat
    for i in range(1, NL):
        wk = conv_w[i - 1]                     # [K, 19, 16]
        o = _base(i)
        bias = conv_b[i].copy()
        vsum = np.zeros(CH, np.float32)
        for k in range(KSIZE):
            comp = io_w[i - 1] @ wk[k][:CH]    # [16(h), 16(out)]
            wbank[:, o + k * 128: o + (k + 1) * 128] = _bd(comp, CH)
            bias += io_b[i - 1] @ wk[k][:CH]
            vsum += wk[k][:CH].sum(axis=0)
        bias += vsum * gam[i - 1]
        bbank[:, i] = np.tile(bias, 8)
        # ctrlS rows: 0-7 S canonical (k=2), 8-15 k=1, 16-23 k=0,
        # 24-31 zero, 32-103 ctrl (32 + b*9 + k*3 + c)
        blk = np.zeros((104, 128), np.float32)
        for b in range(8):
            for k in range(KSIZE):
                vk = wk[k][:CH].sum(axis=0)    # [16]
                for c in range(NUM_CTRL):
                    blk[32 + b * 9 + k * 3 + c, b * 16:(b + 1) * 16] = wk[k][CH + c]
                blk[(KSIZE - 1 - k) * 8 + b, b * 16:(b + 1) * 16] = vk
        wbank[:104, o + 384: o + 512] = blk

    return wbank.astype(BF16), bbank


def _build_per_core_inputs(x_core):
    """x_core: [BL, T, 4] f32 -> (x0 [32, W] bf16, ctrlb [104, 9*W] bf16)."""
    x_core = x_core.astype(np.float32)
    x0 = np.zeros((BL * 4, W), np.float32)
    for b in range(BL):
        x0[b * 4:(b + 1) * 4, :] = x_core[b].T
    ctrl = x_core[:, :, NUM_SIG:]  # [BL, T, 3]
    ctrlb = np.zeros((104, len(UNIQ_DIL) * W), np.float32)
    for di, d in enumerate(UNIQ_DIL):
        for k in range(KSIZE):
            shift = (KSIZE - 1 - k) * d
            for c in range(NUM_CTRL):
                for b in range(BL):
                    r = 32 + b * 9 + k * 3 + c
                    ctrlb[r, di * W + shift: di * W + T] = ctrl[b, : T - shift, c]
    # S~_0 im2col for layer 1 (d=2, block 1), canonical copy in rows 88-95
    s0 = x_core[:, :, 0] / 2.0
    d1 = DILATIONS[1]
    for k in range(KSIZE):
        shift = (KSIZE - 1 - k) * d1
        for b in range(BL):
            ctrlb[(KSIZE - 1 - k) * 8 + b, 1 * W + shift: 1 * W + T] = s0[b, : T - shift]
    return x0.astype(BF16), ctrlb.astype(BF16)


def build_graph():
    nc = bacc.Bacc("TRN2", target_bir_lowering=False, debug=False)

    p_x0 = nc.declare_dram_parameter("x0", [32, W], mybir.dt.bfloat16, isOutput=False)
    p_ctrl = nc.declare_dram_parameter(
        "ctrlb", [104, len(UNIQ_DIL) * W], mybir.dt.bfloat16, isOutput=False)
    p_w = nc.declare_dram_parameter("wbank", [128, NW], mybir.dt.bfloat16, isOutput=False)
    p_b = nc.declare_dram_parameter("bbank", [128, NBIAS], mybir.dt.float32, isOutput=False)
    p_out = nc.declare_dram_parameter("out", [8, FRAME], mybir.dt.float32, isOutput=True)

    with tile.TileContext(nc) as tc:
        with (
            tc.tile_pool(name="persist", bufs=1) as persist,
            tc.tile_pool(name="wu", bufs=1) as wu,
            tc.tile_pool(name="ps", bufs=2, space="PSUM") as ps,
            tc.tile_pool(name="sps", bufs=2, space="PSUM") as sps,
            tc.tile_pool(name="mixp", bufs=1, space="PSUM") as mixp,
        ):
            x0_sb = persist.tile([32, W], mybir.dt.bfloat16, tag="x0")
            ctrl_sb = persist.tile([104, len(UNIQ_DIL) * W], mybir.dt.bfloat16, tag="ctrl")
            w_sb = persist.tile([128, NW], mybir.dt.bfloat16, tag="wbank")
            b_sb = persist.tile([128, NBIAS], mybir.dt.float32, tag="bbank")
            hA = persist.tile([128, W], mybir.dt.bfloat16, tag="hA")
            hB = persist.tile([128, W], mybir.dt.bfloat16, tag="hB")
            out_sb = persist.tile([8, FRAME], mybir.dt.float32, tag="outsb")

            # Front DMAs on the sync HWDGE ring, in deadline order.  Block 1's
            # S rows ship first (layer 0's S-eviction reads them); ctrl blocks
            # for layers 2..9 are loaded mid-graph from the gpsimd queue, after
            # the device S writes to the same block have been emitted.
            nc.sync.dma_start(out=x0_sb[:], in_=p_x0[:])
            nc.sync.dma_start(out=w_sb[:, :392], in_=p_w[:, :392])
            nc.sync.dma_start(out=b_sb[:], in_=p_b[:])
            nc.sync.dma_start(out=ctrl_sb[:24, W:2 * W], in_=p_ctrl[:24, W:2 * W])
            nc.sync.dma_start(out=w_sb[:, 392:1056], in_=p_w[:, 392:1056])
            for i in (2, 3):
                o = _base(i)
                nc.sync.dma_start(out=w_sb[:, o:o + 520], in_=p_w[:, o:o + 520])
            nc.sync.dma_start(out=ctrl_sb[24:104, W:2 * W], in_=p_ctrl[24:104, W:2 * W])
            for i in range(4, NL):
                o = _base(i)
                wid = 520 if i <= 15 else 512
                nc.sync.dma_start(out=w_sb[:, o:o + wid], in_=p_w[:, o:o + wid])

            mixS = mixp.tile([8, FRAME], mybir.dt.float32, tag="mixS")

            # PE warm-up on x0 data (finite, lands first): ramps the PE
            # p-state/HAM clock before layer 0 without a vector dependency.
            for _ in range(12):
                wps = ps.tile([128, TT], mybir.dt.float32, tag="hps")
                nc.tensor.matmul(wps[:, :TT], x0_sb[:32, :128],
                                 x0_sb[:32, :TT], start=True, stop=True)

            pending = []

            def flush(keep=0):
                while len(pending) > keep:
                    for fn in pending.pop(0):
                        fn()

            def o_tap(i, k):
                return k * 128 if i == 0 else _base(i) + k * 128

            for i in range(NL):
                d = DILATIONS[i]
                src_h = x0_sb if i == 0 else (hA if (i - 1) % 2 == 0 else hB)
                dst_h = hA if i % 2 == 0 else hB
                krows = 32 if i == 0 else 128
                lo = LO[i]
                for j in range(lo // TT, NTILES):
                    a = max(j * TT, lo)
                    b = min((j + 1) * TT, T)
                    wid = b - a
                    h_ps = ps.tile([128, TT], mybir.dt.float32, tag="hps")
                    if i == 0:
                        for k in range(KSIZE):
                            shift = (KSIZE - 1 - k) * d
                            nc.tensor.matmul(
                                h_ps[:, :wid],
                                w_sb[:32, k * 128:(k + 1) * 128],
                                x0_sb[:32, a - shift: a - shift + wid],
                                start=(k == 0),
                                stop=(k == KSIZE - 1),
                            )
                    else:
                        for k in range(KSIZE):
                            shift = (KSIZE - 1 - k) * d
                            nc.tensor.matmul(
                                h_ps[:, :wid],
                                w_sb[:, o_tap(i, k): o_tap(i, k) + 128],
                                src_h[:, a - shift: a - shift + wid],
                                start=(k == 0),
                                stop=False,
                            )
                    if i > 0:
                        c0 = _base(i) + 384
                        nc.tensor.matmul(
                            h_ps[:, :wid],
                            w_sb[:104, c0:c0 + 128],
                            ctrl_sb[:104, DI[i] * W + a: DI[i] * W + a + wid],
                            start=False,
                            stop=True,
                        )
                    nc.scalar.activation(
                        out=dst_h[:, a: a + wid],
                        in_=h_ps[:, :wid],
                        func=mybir.ActivationFunctionType.Relu,
                        bias=b_sb[:, i:i + 1],
                        scale=1.0,
                    )
                    flush(keep=2)

                    def epilogue(i=i, j=j, a=a, b=b, dst_h=dst_h):
                        # S-update: S~_{i+1}[t] = u_i.h_i[t] (+ 8 S~_i at evict)
                        if i <= 15 and b > LO[i + 1]:
                            sa = max(a, LO[i + 1])
                            sw = b - sa
                            so = sa - j * TT
                            o_u = O_U0 if i == 0 else _base(i) + 512
                            s_ps = sps.tile([40, TT], mybir.dt.float32, tag="sps")
                            nc.tensor.matmul(
                                s_ps[32:40, so: so + sw],
                                w_sb[:, o_u: o_u + 8],
                                dst_h[:, sa: sa + sw],
                                start=True, stop=True,
                                skip_group_check=True,
                            )
                        if b > MIX_T0:
                            ma = max(a, MIX_T0)
                            mw = b - ma
                            nc.tensor.matmul(
                                mixS[0:8, ma - MIX_T0: ma - MIX_T0 + mw],
                                w_sb[:, O_MIX + i * 8: O_MIX + i * 8 + 8],
                                dst_h[:, ma: ma + mw],
                                start=(i == 0),
                                stop=(i == NL - 1),
                                skip_group_check=True,
                            )
                            # stream the final eviction behind layer 17's
                            # last mix matmuls (chunk complete => evict)
                            if i == NL - 1 and j in (5, 6, 7):
                                c0, c1 = {5: (0, 1024), 6: (1024, 1536),
                                          7: (1536, 2048)}[j]
                                if j == 6:
                                    nc.vector.tensor_scalar_add(
                                        out=out_sb[:, c0:c1],
                                        in0=mixS[0:8, c0:c1],
                                        scalar1=b_sb[:8, NL: NL + 1],
                                    )
                                else:
                                    nc.scalar.activation(
                                        out=out_sb[:, c0:c1],
                                        in_=mixS[0:8, c0:c1],
                                        func=mybir.ActivationFunctionType.Identity,
                                        bias=b_sb[:8, NL: NL + 1],
                                        scale=1.0,
                                    )
                        if i <= 15 and b > LO[i + 1]:
                            sblk = DI[i + 1] * W
                            dblk = DI[i + 2] * W
                            nc.vector.scalar_tensor_tensor(
                                out=ctrl_sb[0:8, dblk + sa: dblk + sa + sw],
                                in0=ctrl_sb[0:8, sblk + sa: sblk + sa + sw],
                                scalar=8.0,
                                in1=s_ps[32:40, so: so + sw],
                                op0=mybir.AluOpType.mult,
                                op1=mybir.AluOpType.add,
                            )

                    pending.append([epilogue])

                if i <= 15:
                    # im2col shift copies of S~_{i+1} for layer i+2's taps,
                    # on the gpsimd engine's DGE queue (separate from sync's).
                    def im2col(i=i):
                        d2 = DILATIONS[i + 2]
                        dblk = DI[i + 2] * W
                        lo2 = LO[i + 2]
                        for k in range(2):
                            shift = (KSIZE - 1 - k) * d2
                            r0 = (KSIZE - 1 - k) * 8
                            nc.gpsimd.dma_start(
                                out=ctrl_sb[r0: r0 + 8,
                                            dblk + lo2: dblk + T],
                                in_=ctrl_sb[0:8,
                                            dblk + lo2 - shift: dblk + T - shift],
                            )

                    pending[-1].append(im2col)
                    if i <= 7:
                        def ctrl_bulk(i=i):
                            di = DI[i + 2]
                            nc.gpsimd.dma_start(
                                out=ctrl_sb[24:104, di * W:(di + 1) * W],
                                in_=p_ctrl[24:104, di * W:(di + 1) * W])

                        pending[-1].append(ctrl_bulk)

            flush()

            # out DMAs (evictions were streamed inside layer-17 epilogues)
            nc.sync.dma_start(out=p_out[:, :1536], in_=out_sb[:, :1536])
            nc.sync.dma_start(out=p_out[:, 1536:], in_=out_sb[:, 1536:])

    nc.finalize()
    return nc


_CACHE = {}


def kernel(**inputs) -> np.ndarray:
    inp = inputs["input"].astype(np.float32)          # [64, 4092, 4]
    wbank, bbank = _build_weight_bank(
        inputs["conv_w0"], inputs["conv_w"], inputs["conv_b"],
        inputs["io_w"], inputs["io_b"], inputs["mixer_w"], inputs["mixer_b"],
    )

    if "nc" not in _CACHE:
        _CACHE["nc"] = build_graph()
    nc = _CACHE["nc"]

    in_maps = []
    for c in range(NCORES):
        x0, ctrlb = _build_per_core_inputs(inp[c * BL:(c + 1) * BL])
        in_maps.append({"x0": x0, "ctrlb": ctrlb, "wbank": wbank, "bbank": bbank})

    res = run_bass_kernel_spmd(nc, in_maps, core_ids=list(range(NCORES)))
    outs = [res.results[c]["out"] for c in range(NCORES)]       # each [8, 2048]
    full = np.concatenate(outs, axis=0)                         # [64, 2048]
    return full[:, :, None].astype(np.float32)                  # [64, 2048, 1]


if __name__ == "__main__":
    data = np.load("/root/problem/ref_inputs.npz")
    out = kernel(**{k: data[k] for k in data.files})
    ref = np.load("/root/problem/ref_out.npy")
    err = np.linalg.norm(out - ref) / np.linalg.norm(ref)
    print("Relative error:", err)


# revision 12
# speedup vs baseline: 1.1762x; 1.0033x over previous
"""Trainium2 Bass kernel for nn_AudioDeviceModel (18-layer dilated causal CNN).

Data parallel over batch (64 = 8 cores x 8).  Per core, (batch, chan) packs
the 128 SBUF partitions and time is the free dim; each conv tap is one
block-diagonal [128,128]x[128,w] TensorEngine matmul with dilation shifts as
free-dim offsets.

The 1x1 "io" mix and the halved channel-sum skip are folded away algebraically:
  sig_{i+1} = io_i(h_i) + io_b_i + S_i,   S_i = sum_c(sig_i)/2
so conv_{i+1} applied to sig_{i+1} becomes composed taps (io_i @ W_k) read
directly from h_i, plus a 1-channel S path with
  S_{i+1} = u_i . h_i + 8 S_i + c_i,  u_i = io_w_i.sum(1)/2.
Constants migrate into conv biases via gamma_{i+1} = 8 gamma_i + c_i.  The S
taps ride in the ctrl matmul (96 contraction rows: 72 ctrl + 24 S-im2col), the
S recursion is one vector op per tile, and the S im2col shifts are SBUF->SBUF
DMA copies issued from the scalar engine's DGE queue.  This removes 2 of 7
matmuls per tile vs the direct formulation.  Epilogue matmuls (S-update, mixer)
lag the conv stream by one tile so ReLU eviction latency stays off the PE
critical path.
"""

import numpy as np
import ml_dtypes

import concourse.bass as bass
import concourse.tile as tile
from concourse import bacc, mybir
from concourse.bass_utils import run_bass_kernel_spmd

# Problem constants (hardcoded; kernel.py must be self-contained).
DILATIONS = [1, 2, 4, 8, 16, 32, 64, 128, 256, 1, 2, 4, 8, 16, 32, 64, 128, 256]
UNIQ_DIL = [1, 2, 4, 8, 16, 32, 64, 128, 256]
DI = [UNIQ_DIL.index(d) for d in DILATIONS]
KSIZE = 3
CH = 16
NUM_SIG = 1
NUM_CTRL = 3
FRAME = 2048
T = 4092
B = 64
NCORES = 8
BL = B // NCORES          # 8 batches per core
W = T                     # no left pad needed: trim bounds never read t<0
NL = len(DILATIONS)       # 18
TT = 512                  # time tile
NTILES = (T + TT - 1) // TT   # 8 (last tile 508 wide)
MIX_T0 = T - FRAME        # 2044: first time index contributing to output

# Receptive-field trim: layer i's output h_i only influences the final frame
# for t >= LO[i]; LO[i] = max(0, LO[i+1] - 2*d_{i+1}) with LO[17] = MIX_T0.
_sums = [0] * NL
_acc = 0
for _i in range(NL - 1, -1, -1):
    _sums[_i] = _acc
    _acc += DILATIONS[_i]
LO = [max(0, MIX_T0 - 2 * _sums[_i]) for _i in range(NL)]

BF16 = ml_dtypes.bfloat16

# Weight bank column layout (bf16, [128, NW]), in DMA-stream order:
#   0:384     layer-0 taps (rows :32)
#   384:392   u_0
#   392:536   mixer blocks (18 x 8)
#   base(i) = 536 + (i-1)*520 for i=1..15: taps(384) | ctrlS(128, rows :104) | u_i(8)
#   i=16,17: 512-wide blocks (no u)
def _base(i):
    assert i >= 1
    return 536 + (i - 1) * 520 if i <= 16 else 536 + 15 * 520 + 512

NW = 536 + 15 * 520 + 512 + 512       # 9360
O_U0 = 384
O_MIX = 392
NBIAS = NL + 1                        # 18 conv biases | mixer_b


def _bd(block, k_per_b):
    # block: [k_per_b, 16] -> [8*k_per_b, 128] block diagonal over batches
    m = np.zeros((8 * k_per_b, 128), np.float32)
    for b in range(8):
        m[b * k_per_b:(b + 1) * k_per_b, b * 16:(b + 1) * 16] = block
    return m


def _build_weight_bank(conv_w0, conv_w, conv_b, io_w, io_b, mixer_w, mixer_b):
    conv_w0 = conv_w0.astype(np.float32)
    conv_w = conv_w.astype(np.float32)
    conv_b = conv_b.astype(np.float32)
    io_w = io_w.astype(np.float32)
    io_b = io_b.astype(np.float32)
    mixer_w = mixer_w.astype(np.float32)
    mixer_b = mixer_b.astype(np.float32)

    wbank = np.zeros((128, NW), np.float32)
    bbank = np.zeros((128, NBIAS), np.float32)

    # gamma_i: S_i = S~_i + gamma_i
    gam = [0.0] * NL
    for i in range(1, NL):
        gam[i] = 8.0 * gam[i - 1] + io_b[i - 1].sum() / 2.0

    # layer 0: raw taps on [sig|ctrl] (4 ch/batch)
    for k in range(KSIZE):
        wbank[:32, k * 128:(k + 1) * 128] = _bd(conv_w0[k], 4)
    bbank[:, 0] = np.tile(conv_b[0], 8)

    # u_i blocks (i = 0..15)
    for i in range(16):
        u = io_w[i].sum(axis=1) / 2.0          # [16]
        o = O_U0 if i == 0 else _base(i) + 512
        for b in range(8):
            wbank[b * 16:(b + 1) * 16, o + b] = u

    # mixer blocks
    for i in range(NL):
        for b in range(8):
            wbank[b * 16:(b + 1) * 16, O_MIX + i * 8 + b] = \
                mixer_w[i * CH:(i + 1) * CH, 0]
    bbank[:8, NL] = mixer_b[0]

    # layers 1..17: composed taps + ctrlS block + bias-hat
    for i in range(1, NL):
        wk = conv_w[i - 1]                     # [K, 19, 16]
        o = _base(i)
        bias = conv_b[i].copy()
        vsum = np.zeros(CH, np.float32)
        for k in range(KSIZE):
            comp = io_w[i - 1] @ wk[k][:CH]    # [16(h), 16(out)]
            wbank[:, o + k * 128: o + (k + 1) * 128] = _bd(comp, CH)
            bias += io_b[i - 1] @ wk[k][:CH]
            vsum += wk[k][:CH].sum(axis=0)
        bias += vsum * gam[i - 1]
        bbank[:, i] = np.tile(bias, 8)
        # ctrlS rows: 0-7 S canonical (k=2), 8-15 k=1, 16-23 k=0,
        # 24-31 zero, 32-103 ctrl (32 + b*9 + k*3 + c)
        blk = np.zeros((104, 128), np.float32)
        for b in range(8):
            for k in range(KSIZE):
                vk = wk[k][:CH].sum(axis=0)    # [16]
                for c in range(NUM_CTRL):
                    blk[32 + b * 9 + k * 3 + c, b * 16:(b + 1) * 16] = wk[k][CH + c]
                blk[(KSIZE - 1 - k) * 8 + b, b * 16:(b + 1) * 16] = vk
        wbank[:104, o + 384: o + 512] = blk

    return wbank.astype(BF16), bbank


def _build_per_core_inputs(x_core):
    """x_core: [BL, T, 4] f32 -> (x0 [32, W] bf16, ctrlb [104, 9*W] bf16)."""
    x_core = x_core.astype(np.float32)
    x0 = np.zeros((BL * 4, W), np.float32)
    for b in range(BL):
        x0[b * 4:(b + 1) * 4, :] = x_core[b].T
    ctrl = x_core[:, :, NUM_SIG:]  # [BL, T, 3]
    ctrlb = np.zeros((104, len(UNIQ_DIL) * W), np.float32)
    for di, d in enumerate(UNIQ_DIL):
        for k in range(KSIZE):
            shift = (KSIZE - 1 - k) * d
            for c in range(NUM_CTRL):
                for b in range(BL):
                    r = 32 + b * 9 + k * 3 + c
                    ctrlb[r, di * W + shift: di * W + T] = ctrl[b, : T - shift, c]
    # S~_0 im2col for layer 1 (d=2, block 1), canonical copy in rows 88-95
    s0 = x_core[:, :, 0] / 2.0
    d1 = DILATIONS[1]
    for k in range(KSIZE):
        shift = (KSIZE - 1 - k) * d1
        for b in range(BL):
            ctrlb[(KSIZE - 1 - k) * 8 + b, 1 * W + shift: 1 * W + T] = s0[b, : T - shift]
    return x0.astype(BF16), ctrlb.astype(BF16)


def build_graph():
    nc = bacc.Bacc("TRN2", target_bir_lowering=False, debug=False)

    p_x0 = nc.declare_dram_parameter("x0", [32, W], mybir.dt.bfloat16, isOutput=False)
    p_ctrl = nc.declare_dram_parameter(
        "ctrlb", [104, len(UNIQ_DIL) * W], mybir.dt.bfloat16, isOutput=False)
    p_w = nc.declare_dram_parameter("wbank", [128, NW], mybir.dt.bfloat16, isOutput=False)
    p_b = nc.declare_dram_parameter("bbank", [128, NBIAS], mybir.dt.float32, isOutput=False)
    p_out = nc.declare_dram_parameter("out", [8, FRAME], mybir.dt.float32, isOutput=True)

    with tile.TileContext(nc) as tc:
        with (
            tc.tile_pool(name="persist", bufs=1) as persist,
            tc.tile_pool(name="wu", bufs=1) as wu,
            tc.tile_pool(name="ps", bufs=2, space="PSUM") as ps,
            tc.tile_pool(name="sps", bufs=2, space="PSUM") as sps,
            tc.tile_pool(name="mixp", bufs=1, space="PSUM") as mixp,
        ):
            x0_sb = persist.tile([32, W], mybir.dt.bfloat16, tag="x0")
            ctrl_sb = persist.tile([104, len(UNIQ_DIL) * W], mybir.dt.bfloat16, tag="ctrl")
            w_sb = persist.tile([128, NW], mybir.dt.bfloat16, tag="wbank")
            b_sb = persist.tile([128, NBIAS], mybir.dt.float32, tag="bbank")
            hA = persist.tile([128, W], mybir.dt.bfloat16, tag="hA")
            hB = persist.tile([128, W], mybir.dt.bfloat16, tag="hB")
            out_sb = persist.tile([8, FRAME], mybir.dt.float32, tag="outsb")

            # Front DMAs on the sync HWDGE ring, in deadline order.  Block 1's
            # S rows ship first (layer 0's S-eviction reads them); ctrl blocks
            # for layers 2..9 are loaded mid-graph from the gpsimd queue, after
            # the device S writes to the same block have been emitted.
            nc.sync.dma_start(out=x0_sb[:16], in_=p_x0[:16])
            nc.scalar.dma_start(out=x0_sb[16:32], in_=p_x0[16:32])
            nc.sync.dma_start(out=w_sb[:, :392], in_=p_w[:, :392])
            nc.sync.dma_start(out=b_sb[:], in_=p_b[:])
            nc.scalar.dma_start(out=ctrl_sb[:24, W:2 * W], in_=p_ctrl[:24, W:2 * W])
            nc.sync.dma_start(out=w_sb[:, 392:1056], in_=p_w[:, 392:1056])
            for i in (2, 3):
                o = _base(i)
                nc.sync.dma_start(out=w_sb[:, o:o + 520], in_=p_w[:, o:o + 520])
            nc.sync.dma_start(out=ctrl_sb[24:104, W:2 * W], in_=p_ctrl[24:104, W:2 * W])
            for i in range(4, NL):
                o = _base(i)
                wid = 520 if i <= 15 else 512
                nc.sync.dma_start(out=w_sb[:, o:o + wid], in_=p_w[:, o:o + wid])

            mixS = mixp.tile([8, FRAME], mybir.dt.float32, tag="mixS")

            # PE warm-up reading the first x0 half (earliest DMA to land):
            # ramps the PE p-state/HAM clock before layer 0.
            for _ in range(8):
                wps = ps.tile([128, TT], mybir.dt.float32, tag="hps")
                nc.tensor.matmul(wps[:, :TT], x0_sb[:16, :128],
                                 x0_sb[:16, :TT], start=True, stop=True)

            pending = []

            def flush(keep=0):
                while len(pending) > keep:
                    for fn in pending.pop(0):
                        fn()

            def o_tap(i, k):
                return k * 128 if i == 0 else _base(i) + k * 128

            for i in range(NL):
                d = DILATIONS[i]
                src_h = x0_sb if i == 0 else (hA if (i - 1) % 2 == 0 else hB)
                dst_h = hA if i % 2 == 0 else hB
                krows = 32 if i == 0 else 128
                lo = LO[i]
                for j in range(lo // TT, NTILES):
                    a = max(j * TT, lo)
                    b = min((j + 1) * TT, T)
                    wid = b - a
                    h_ps = ps.tile([128, TT], mybir.dt.float32, tag="hps")
                    if i == 0:
                        for k in range(KSIZE):
                            shift = (KSIZE - 1 - k) * d
                            nc.tensor.matmul(
                                h_ps[:, :wid],
                                w_sb[:32, k * 128:(k + 1) * 128],
                                x0_sb[:32, a - shift: a - shift + wid],
                                start=(k == 0),
                                stop=(k == KSIZE - 1),
                            )
                            if k == 0:
                                flush(keep=1)
                    else:
                        for k in range(KSIZE):
                            shift = (KSIZE - 1 - k) * d
                            nc.tensor.matmul(
                                h_ps[:, :wid],
                                w_sb[:, o_tap(i, k): o_tap(i, k) + 128],
                                src_h[:, a - shift: a - shift + wid],
                                start=(k == 0),
                                stop=False,
                            )
                            if k == 0:
                                flush(keep=1)
                    if i > 0:
                        c0 = _base(i) + 384
                        nc.tensor.matmul(
                            h_ps[:, :wid],
                            w_sb[:104, c0:c0 + 128],
                            ctrl_sb[:104, DI[i] * W + a: DI[i] * W + a + wid],
                            start=False,
                            stop=True,
                        )
                    nc.scalar.activation(
                        out=dst_h[:, a: a + wid],
                        in_=h_ps[:, :wid],
                        func=mybir.ActivationFunctionType.Relu,
                        bias=b_sb[:, i:i + 1],
                        scale=1.0,
                    )

                    def epilogue(i=i, j=j, a=a, b=b, dst_h=dst_h):
                        # S-update: S~_{i+1}[t] = u_i.h_i[t] (+ 8 S~_i at evict)
                        if i <= 15 and b > LO[i + 1]:
                            sa = max(a, LO[i + 1])
                            sw = b - sa
                            so = sa - j * TT
                            o_u = O_U0 if i == 0 else _base(i) + 512
                            s_ps = sps.tile([40, TT], mybir.dt.float32, tag="sps")
                            nc.tensor.matmul(
                                s_ps[32:40, so: so + sw],
                                w_sb[:, o_u: o_u + 8],
                                dst_h[:, sa: sa + sw],
                                start=True, stop=True,
                                skip_group_check=True,
                            )
                        if b > MIX_T0:
                            ma = max(a, MIX_T0)
                            mw = b - ma
                            nc.tensor.matmul(
                                mixS[0:8, ma - MIX_T0: ma - MIX_T0 + mw],
                                w_sb[:, O_MIX + i * 8: O_MIX + i * 8 + 8],
                                dst_h[:, ma: ma + mw],
                                start=(i == 0),
                                stop=(i == NL - 1),
                                skip_group_check=True,
                            )
                            # stream the final eviction behind layer 17's
                            # last mix matmuls (chunk complete => evict)
                            if i == NL - 1 and j in (5, 6, 7):
                                c0, c1 = {5: (0, 1024), 6: (1024, 1536),
                                          7: (1536, 2048)}[j]
                                if j == 6:
                                    nc.vector.tensor_scalar_add(
                                        out=out_sb[:, c0:c1],
                                        in0=mixS[0:8, c0:c1],
                                        scalar1=b_sb[:8, NL: NL + 1],
                                    )
                                else:
                                    nc.scalar.activation(
                                        out=out_sb[:, c0:c1],
                                        in_=mixS[0:8, c0:c1],
                                        func=mybir.ActivationFunctionType.Identity,
                                        bias=b_sb[:8, NL: NL + 1],
                                        scale=1.0,
                                    )
                        if i <= 15 and b > LO[i + 1]:
                            sblk = DI[i + 1] * W
                            dblk = DI[i + 2] * W
                            nc.vector.scalar_tensor_tensor(
                                out=ctrl_sb[0:8, dblk + sa: dblk + sa + sw],
                                in0=ctrl_sb[0:8, sblk + sa: sblk + sa + sw],
                                scalar=8.0,
                                in1=s_ps[32:40, so: so + sw],
                                op0=mybir.AluOpType.mult,
                                op1=mybir.AluOpType.add,
                            )

                    pending.append([epilogue])

                if i <= 15:
                    # im2col shift copies of S~_{i+1} for layer i+2's taps,
                    # on the gpsimd engine's DGE queue (separate from sync's).
                    def im2col(i=i):
                        d2 = DILATIONS[i + 2]
                        dblk = DI[i + 2] * W
                        lo2 = LO[i + 2]
                        for k in range(2):
                            shift = (KSIZE - 1 - k) * d2
                            r0 = (KSIZE - 1 - k) * 8
                            nc.gpsimd.dma_start(
                                out=ctrl_sb[r0: r0 + 8,
                                            dblk + lo2: dblk + T],
                                in_=ctrl_sb[0:8,
                                            dblk + lo2 - shift: dblk + T - shift],
                            )

                    pending[-1].append(im2col)
                    if i <= 7:
                        def ctrl_bulk(i=i):
                            di = DI[i + 2]
                            nc.gpsimd.dma_start(
                                out=ctrl_sb[24:104, di * W:(di + 1) * W],
                                in_=p_ctrl[24:104, di * W:(di + 1) * W])

                        pending[-1].append(ctrl_bulk)

            flush()

            # out DMAs (evictions were streamed inside layer-17 epilogues)
            nc.sync.dma_start(out=p_out[:, :1536], in_=out_sb[:, :1536])
            nc.sync.dma_start(out=p_out[:, 1536:], in_=out_sb[:, 1536:])

    nc.finalize()
    return nc


_CACHE = {}


def kernel(**inputs) -> np.ndarray:
    inp = inputs["input"].astype(np.float32)          # [64, 4092, 4]
    wbank, bbank = _build_weight_bank(
        inputs["conv_w0"], inputs["conv_w"], inputs["conv_b"],
        inputs["io_w"], inputs["io_b"], inputs["mixer_w"], inputs["mixer_b"],
    )

    if "nc" not in _CACHE:
        _CACHE["nc"] = build_graph()
    nc = _CACHE["nc"]

    in_maps = []
    for c in range(NCORES):
        x0, ctrlb = _build_per_core_inputs(inp[c * BL:(c + 1) * BL])
        in_maps.append({"x0": x0, "ctrlb": ctrlb, "wbank": wbank, "bbank": bbank})

    res = run_bass_kernel_spmd(nc, in_maps, core_ids=list(range(NCORES)))
    outs = [res.results[c]["out"] for c in range(NCORES)]       # each [8, 2048]
    full = np.concatenate(outs, axis=0)                         # [64, 2048]
    return full[:, :, None].astype(np.float32)                  # [64, 2048, 1]


if __name__ == "__main__":
    data = np.load("/root/problem/ref_inputs.npz")
    out = kernel(**{k: data[k] for k in data.files})
    ref = np.load("/root/problem/ref_out.npy")
    err = np.linalg.norm(out - ref) / np.linalg.norm(ref)
    print("Relative error:", err)
